# revision 34
# baseline (speedup 1.0000x reference)
"""Trainium2 Bass kernel for CrossDepthAttentionResidual.

Reference computation (L=12, B=2, S=2048, D=1024, DK=256):
    normalized = LayerNorm_D(states)                    # (L,B,S,D)
    query  = normalized[-1] @ Wq.T                      # (B,S,DK)
    keys   = normalized @ Wk.T                          # (L,B,S,DK)
    logits = einsum('bsk,lbsk->lbs', query, keys)/16    # (L,B,S)
    w      = softmax_l(logits)
    mixed  = einsum('lbs,lbsd->bsd', w, states)
    out    = g*states[-1] + (1-g)*mixed,  g = sigmoid(latest_gate)

Key algebraic rewrite (non-affine LN, w==1 b==0):
    logits[l,n] = u[n] . norm[l,n],  u[n] = Wk.T Wq norm11[n]
Expanding norm11 = (x11 - mu11) r11:
    u[n] = r11[n] u'[n],   u'[n] = M x11[n] - mu11[n] s,  s = M 1,  M = Wk.T Wq
u' is computed ENTIRELY on the TensorEngine from RAW x11 (no LayerNorm on
the critical path): transpose -> q' = Wq x11^T (tile-PAIR batched so the
matmuls run 256 cols wide) -> v = Wk^T q' accumulated in PSUM, plus a
rank-1 matmul (-mu11^T (x) s) into the same PSUM region, so u' falls out of
the PSUM->SBUF copy whose accum_out is C1 = sum_d u'.  r11 never touches a
big tensor: logits[l,n] = rr2_l A[l,n] - rr2_l mu_l C1[n] with
rr2_l = rsqrt((var_l+eps)(var11+eps)) (one fused Newton rsqrt, single
iteration), A[l,n] = u'[n].x[l,n].  Softmax over depth skips
max-subtraction (exponents are bounded by |q||k|/16, empirically ~2).

Engine balance per 128-position tile (cost-model ns): stats for 7 layers on
ACT (Copy+Square w/ accum, 2448/layer), 4+x11 on DVE (bn_stats, 1261);
all 12 A-dots are fused DVE scalar_tensor_tensor+accum (1127); dg diag prep
+ softmax on DVE; mix + transposes + q'/v/rank-1 on PE; output stores go
out the ACT HWDGE queue so they never head-block the SP load queue.
GPSIMD (Pool) is intentionally UNUSED: on real HW it has ~1us
per-instruction overhead and far-below-model bulk throughput.

Sharding: positions (b*S+s) split contiguously across 8 cores; all compute is
pointwise in position, so no collectives are needed.
"""

import math
from contextlib import ExitStack

import numpy as np

import concourse.bacc as bacc
import concourse.mybir as mybir
import concourse.tile as tile
from concourse import masks
from concourse.bass_utils import run_bass_kernel_spmd

L, B, S, D, DK = 12, 2, 2048, 1024, 256
N_CORES = 8
NTOT = B * S            # 4096 positions
NPC = NTOT // N_CORES   # 512 positions per core
P = 128                 # SBUF partitions
LN_EPS = 1e-5
SCALE = 1.0 / math.sqrt(DK)
ACT_L0 = 4
ACT_STAT = frozenset(range(4, 11))  # stats via ACT Copy/Square+accum

F32 = mybir.dt.float32
F32R = mybir.dt.float32r
BF16 = mybir.dt.bfloat16
U32 = mybir.dt.uint32
ALU = mybir.AluOpType
ACTF = mybir.ActivationFunctionType

RSQRT_MAGIC = 0x5F3759DF


def _rsqrt_newton(nc, pool, vpe, r_out, ncols, n_iter=2):
    """r_out = rsqrt(vpe) via bit-trick seed + Newton iterations (pure DVE).

    Avoids the ScalarEngine Sqrt table set (2.7us table switch + 65536-ULP
    budget).  vpe, r_out: [128, ncols] f32 SBUF tiles (contiguous).
    """
    magic = pool.tile([P, ncols], U32, tag="rs_magic")
    nc.vector.memset(magic[:], RSQRT_MAGIC)
    shifted = pool.tile([P, ncols], U32, tag="rs_shift")
    nc.vector.tensor_scalar(
        out=shifted[:], in0=vpe[:].bitcast(U32), scalar1=1, scalar2=None,
        op0=ALU.logical_shift_right,
    )
    yu = pool.tile([P, ncols], U32, tag="rs_seed")
    nc.vector.tensor_tensor(out=yu[:], in0=magic[:], in1=shifted[:], op=ALU.subtract)
    y = yu[:].bitcast(F32)
    t = pool.tile([P, ncols], F32, tag="rs_tmp")
    for _ in range(n_iter):
        # y <- y * (1.5 - 0.5 * vpe * y^2)
        nc.vector.tensor_tensor(out=t[:], in0=y, in1=y, op=ALU.mult)
        nc.vector.tensor_tensor(out=t[:], in0=t[:], in1=vpe[:], op=ALU.mult)
        nc.vector.tensor_scalar(
            out=t[:], in0=t[:], scalar1=-0.5, scalar2=1.5, op0=ALU.mult, op1=ALU.add,
        )
        nc.vector.tensor_tensor(out=t[:], in0=y, in1=t[:], op=ALU.mult)
        nc.vector.tensor_copy(r_out[:], t[:])
    return r_out


def build_program(npc, gate, use_affine, bench_loop=0):
    """Build the per-core SPMD Bass program.

    npc: positions handled by this core (multiple of 128).
    gate: float python scalar sigmoid(latest_gate), baked as immediates.
    use_affine: general ln_weight/ln_bias path (falls back to v1 program).
    bench_loop: if > 0, wrap the whole body in a hardware loop that repeats
        it bench_loop times (for timing measurements only).
    """
    if use_affine:
        return _build_program_v1(npc, gate, use_affine, bench_loop)
    assert npc % P == 0
    nt = npc // P
    g = float(gate)

    nc = bacc.Bacc("TRN2", target_bir_lowering=False, debug=False)

    x_dram = nc.dram_tensor("states_shard", [L, npc, D], F32R, kind="ExternalInput")
    # wqt: [128, 8*256]; chunk c cols [c*256,(c+1)*256) holds Wq.T[c*128:(c+1)*128, :]
    wqt_dram = nc.dram_tensor("wqt", [P, 8 * DK], F32R, kind="ExternalInput")
    # wk: [128, 2*1024]; chunk h cols [h*1024,...) holds Wk[h*128:(h+1)*128, :]
    wk_dram = nc.dram_tensor("wk", [P, 2 * D], F32R, kind="ExternalInput")
    # svec: [1, 1024] = Wk.T @ (Wq @ ones) = M @ 1
    s_dram = nc.dram_tensor("svec", [1, D], F32R, kind="ExternalInput")
    out_dram = nc.dram_tensor("out", [npc, D], F32, kind="ExternalOutput")

    with tile.TileContext(nc) as tc, ExitStack() as ctx:
        cpool = ctx.enter_context(tc.tile_pool(name="consts", bufs=1))
        gpool = ctx.enter_context(tc.tile_pool(name="globals", bufs=1))
        epool = ctx.enter_context(tc.tile_pool(name="x11", bufs=4))
        xpool = ctx.enter_context(tc.tile_pool(name="x", bufs=2))
        tpool = ctx.enter_context(tc.tile_pool(name="tposed", bufs=2))
        qpool = ctx.enter_context(tc.tile_pool(name="q", bufs=2))
        mpool = ctx.enter_context(tc.tile_pool(name="negmuT", bufs=2))
        upool = ctx.enter_context(tc.tile_pool(name="u", bufs=2))
        bpool = ctx.enter_context(tc.tile_pool(name="dump", bufs=3))
        dpool = ctx.enter_context(tc.tile_pool(name="dotdump", bufs=3))
        spool = ctx.enter_context(tc.tile_pool(name="stats", bufs=2))
        dgpool = ctx.enter_context(tc.tile_pool(name="dg", bufs=3))
        opool = ctx.enter_context(tc.tile_pool(name="osb", bufs=2))
        pT = ctx.enter_context(tc.tile_pool(name="psum_T", bufs=1, space="PSUM"))
        pQ = ctx.enter_context(tc.tile_pool(name="psum_q", bufs=1, space="PSUM"))
        pU = ctx.enter_context(tc.tile_pool(name="psum_u", bufs=1, space="PSUM"))
        pM = ctx.enter_context(tc.tile_pool(name="psum_m", bufs=2, space="PSUM"))

        # ---- constants ----
        wqt = cpool.tile([P, 8 * DK], F32R)
        nc.sync.dma_start(wqt[:], wqt_dram[:])
        wk = cpool.tile([P, 2 * D], F32R)
        nc.sync.dma_start(wk[:], wk_dram[:])
        s_row = cpool.tile([1, D], F32R)
        nc.sync.dma_start(s_row[:], s_dram[:])
        ident_f = cpool.tile([P, P], F32)
        masks.make_identity(nc, ident_f[:])
        ident = cpool.tile([P, P], F32R)
        nc.scalar.copy(ident[:], ident_f[:])

        loop_ctx = tc.For_i(0, bench_loop, 1) if bench_loop > 0 else None
        if loop_ctx is not None:
            ctx.enter_context(loop_ctx)

        # ---- persistent per-core state (small) ----
        st_all = gpool.tile([P, nt, L, 12], F32)  # bn_stats scratch
        ag_all = gpool.tile([P, nt, L, 2], F32)   # [mean, var] per (tile, layer)
        av_all = gpool.tile([P, nt, L], F32)      # A dots (u'.x)
        sx_all = gpool.tile([P, nt, len(ACT_STAT)], F32)
        sxx_all = gpool.tile([P, nt, len(ACT_STAT)], F32)
        c1_all = gpool.tile([P, nt], F32)         # sum_d u'
        vpe11_all = gpool.tile([P, nt], F32)      # var11 + eps

        GF = g / (1.0 - g)
        u_sbs = {}

        # x11 (layer 11) for the first two tiles loads up front so phase A
        # of tile t+1 runs entirely under tile t's layer loads
        x11s = []
        for tt in range(min(2, nt)):
            x11n = epool.tile([P, D], F32R, tag="x11")
            for hh in range(2):
                nc.sync.dma_start(
                    x11n[:, hh * 512:(hh + 1) * 512],
                    x_dram[L - 1, tt * P:(tt + 1) * P, hh * 512:(hh + 1) * 512])
            x11s.append(x11n)

        for t in range(nt):
            r0 = t * P
            x11 = x11s[t]
            xls = []
            for l in range(L - 1):
                xt = xpool.tile([P, D], F32R, tag="xl", bufs=28)
                xl = xt[:]
                nc.sync.dma_start(xl, x_dram[l, r0:r0 + P, :])
                xls.append(xl)
            if t + 2 < nt:
                x11n = epool.tile([P, D], F32R, tag="x11")
                for hh in range(2):
                    nc.sync.dma_start(
                        x11n[:, hh * 512:(hh + 1) * 512],
                        x_dram[L - 1, (t + 2) * P:(t + 3) * P,
                               hh * 512:(hh + 1) * 512])
                x11s.append(x11n)

            # ====== Phase A: u' on the TensorEngine from raw x11, emitted
            # for tile PAIRS (t, t+1) so the q' matmuls run 256 cols wide ====
            if t % 2 == 0:
                pair = [t] if t + 1 >= nt else [t, t + 1]
                np_ = len(pair)
                with tc.high_priority():
                    # transpose x11 tiles -> x11t [d, (tile, pos)]
                    x11t = tpool.tile([P, np_, D], F32R, tag="x11t")
                    for j, tj in enumerate(pair):
                        for half in range(2):
                            pt = pT.tile([P, 512], F32R, tag="pT")
                            for cc in range(4):
                                c = half * 4 + cc
                                nc.tensor.transpose(
                                    pt[:, cc * P:(cc + 1) * P],
                                    x11s[tj][:, c * P:(c + 1) * P], ident[:])
                            nc.scalar.copy(
                                x11t[:, j, half * 512:(half + 1) * 512], pt[:])
                    negmus = []
                    for j, tj in enumerate(pair):
                        nc.vector.bn_stats(st_all[:, tj, L - 1, 0:6],
                                           x11s[tj][:, 0:512].bitcast(F32))
                        nc.vector.bn_stats(st_all[:, tj, L - 1, 6:12],
                                           x11s[tj][:, 512:1024].bitcast(F32))
                        nc.vector.bn_aggr(ag_all[:, tj, L - 1, :],
                                          st_all[:, tj, L - 1, :])
                        nc.vector.tensor_scalar(out=vpe11_all[:, tj:tj + 1],
                                                in0=ag_all[:, tj, L - 1, 1:2],
                                                scalar1=LN_EPS, scalar2=None,
                                                op0=ALU.add)
                        negmu = spool.tile([P, 1], F32R, tag=f"negmu{j}")
                        nc.vector.tensor_scalar(out=negmu[:],
                                                in0=ag_all[:, tj, L - 1, 0:1],
                                                scalar1=-1.0, scalar2=None,
                                                op0=ALU.mult)
                        negmus.append(negmu)
                    # q' = Wq x11t for the pair (rhs 256 cols wide)
                    pq = pQ.tile([P, 2, np_ * P], F32, tag="pq")
                    for h in range(2):
                        for c in range(8):
                            nc.tensor.matmul(
                                pq[:, h, :],
                                lhsT=wqt[:, c * DK + h * P: c * DK + (h + 1) * P]
                                    .bitcast(F32R),
                                rhs=x11t[:, :, c * P:(c + 1) * P].bitcast(F32R),
                                start=(c == 0), stop=(c == 7),
                            )
                    qsb = qpool.tile([P, 2, np_ * P], F32R, tag="qsb")
                    nc.scalar.copy(qsb[:], pq[:])
                    # -mu11^T via PE transpose, reusing the q' PSUM after copy
                    negmuTs = []
                    for j, tj in enumerate(pair):
                        nc.tensor.matmul(pq[0:1, 0, j * P:(j + 1) * P],
                                         lhsT=negmus[j][:],
                                         rhs=ident[:], start=True, stop=True)
                        negmuT = mpool.tile([1, P], F32R, tag=f"negmuT{j}")
                        nc.vector.tensor_copy(negmuT[:],
                                              pq[0:1, 0, j * P:(j + 1) * P])
                        negmuTs.append(negmuT)
                    # u' = Wk.T q' - mu11 (x) s per tile, rank-1 in PSUM
                    for j, tj in enumerate(pair):
                        pu = pU.tile([P, D], F32, tag="pu")
                        for nh in range(2):
                            for h in range(2):
                                nc.tensor.matmul(
                                    pu[:, nh * 512:(nh + 1) * 512],
                                    lhsT=qsb[:, h, j * P:(j + 1) * P]
                                        .bitcast(F32R),
                                    rhs=wk[:, h * D + nh * 512:
                                           h * D + (nh + 1) * 512]
                                        .bitcast(F32R),
                                    start=(h == 0), stop=False,
                                )
                            nc.tensor.matmul(
                                pu[:, nh * 512:(nh + 1) * 512],
                                lhsT=negmuTs[j][:],
                                rhs=s_row[:, nh * 512:(nh + 1) * 512],
                                start=False, stop=True,
                            )
                        u_sb = upool.tile([P, D], F32, tag="u_sb")
                        nc.scalar.activation(out=u_sb[:], in_=pu[:],
                                             func=ACTF.Copy,
                                             accum_out=c1_all[:, tj:tj + 1])
                        u_sbs[tj] = u_sb
            u_sb = u_sbs[t]

            # ================= Phase B(t): stats + dots =================
            # stats: ACT Copy+Square for ACT_STAT layers, DVE bn for the rest
            for l in range(L - 1):
                if l in ACT_STAT:
                    dc = bpool.tile([P, D], BF16, tag="dump")
                    nc.scalar.activation(out=dc[:], in_=xls[l].bitcast(F32),
                                         func=ACTF.Copy,
                                         accum_out=sx_all[:, t, l - ACT_L0:
                                                          l - ACT_L0 + 1])
                    ds = bpool.tile([P, D], BF16, tag="dump")
                    nc.scalar.activation(out=ds[:], in_=xls[l].bitcast(F32),
                                         func=ACTF.Square,
                                         accum_out=sxx_all[:, t, l - ACT_L0:
                                                           l - ACT_L0 + 1])
                else:
                    nc.vector.bn_stats(st_all[:, t, l, 0:6],
                                       xls[l][:, 0:512].bitcast(F32))
                    nc.vector.bn_stats(st_all[:, t, l, 6:12],
                                       xls[l][:, 512:1024].bitcast(F32))
                    nc.vector.bn_aggr(ag_all[:, t, l, :], st_all[:, t, l, :])
            # mean/var for the ACT-stat layers from the raw sums (on Pool)
            NA = len(ACT_STAT)
            tma = spool.tile([P, NA], F32, tag="tma")
            nc.vector.tensor_scalar(out=ag_all[:, t, ACT_L0:ACT_L0 + NA, 0],
                                    in0=sx_all[:, t, :],
                                    scalar1=1.0 / D, scalar2=None,
                                    op0=ALU.mult)
            mean_ap = ag_all[:, t, ACT_L0:ACT_L0 + NA, 0]
            nc.vector.tensor_tensor(out=tma[:], in0=mean_ap, in1=mean_ap,
                                    op=ALU.mult)
            nc.vector.scalar_tensor_tensor(
                out=ag_all[:, t, ACT_L0:ACT_L0 + NA, 1],
                in0=sxx_all[:, t, :], scalar=1.0 / D, in1=tma[:],
                op0=ALU.mult, op1=ALU.subtract)
            # A[l] = u'.x_l : l=11 first (x11 is resident), then fused DVE
            # stt dots chasing the layer loads
            pr = dpool.tile([P, D], BF16, tag="pr")
            nc.vector.scalar_tensor_tensor(
                out=pr[:], in0=x11[:].bitcast(F32), scalar=1.0, in1=u_sb[:],
                op0=ALU.mult, op1=ALU.mult,
                accum_out=av_all[:, t, L - 1:L])
            for l in range(L - 1):
                pr = dpool.tile([P, D], BF16, tag="pr")
                nc.vector.scalar_tensor_tensor(
                    out=pr[:], in0=xls[l].bitcast(F32), scalar=1.0,
                    in1=u_sb[:], op0=ALU.mult, op1=ALU.mult,
                    accum_out=av_all[:, t, l:l + 1])

            # ---- logits + softmax + gate fold (v1-style single batch) ----
            vpe = spool.tile([P, L], F32, tag="vpe")
            nc.vector.tensor_scalar(out=vpe[:],
                                    in0=ag_all[:, t, :, 1],
                                    scalar1=LN_EPS,
                                    scalar2=vpe11_all[:, t:t + 1],
                                    op0=ALU.add, op1=ALU.mult)
            rr2 = spool.tile([P, L], F32, tag="rr2")
            _rsqrt_newton(nc, spool, vpe, rr2, L, n_iter=1)
            lg = spool.tile([P, L], F32, tag="lg")
            nc.vector.tensor_tensor(out=lg[:], in0=av_all[:, t, :],
                                    in1=rr2[:], op=ALU.mult)
            mur = spool.tile([P, L], F32, tag="mur")
            nc.vector.tensor_tensor(out=mur[:], in0=ag_all[:, t, :, 0],
                                    in1=rr2[:], op=ALU.mult)
            nc.vector.tensor_scalar(out=mur[:], in0=mur[:],
                                    scalar1=c1_all[:, t:t + 1],
                                    scalar2=None, op0=ALU.mult)
            nc.vector.tensor_tensor(out=lg[:], in0=lg[:], in1=mur[:],
                                    op=ALU.subtract)
            wts = spool.tile([P, L], F32, tag="wts")
            ssum = spool.tile([P, 1], F32, tag="ssum")
            nc.scalar.activation(
                out=wts[:], in_=lg[:], func=ACTF.Exp, scale=SCALE,
                accum_out=ssum[:],
            )
            rs = spool.tile([P, 1], F32, tag="rs")
            nc.vector.reciprocal(rs[:], ssum[:])
            nc.vector.tensor_scalar(out=rs[:], in0=rs[:], scalar1=(1.0 - g),
                                    scalar2=None, op0=ALU.mult)
            nc.vector.tensor_scalar(out=wts[:], in0=wts[:], scalar1=rs[:],
                                    scalar2=None, op0=ALU.mult)
            nc.vector.tensor_scalar(out=wts[:, L - 1:L], in0=wts[:, L - 1:L],
                                    scalar1=g, scalar2=None, op0=ALU.add)

            pm = pM.tile([P, D], F32, tag="pm")
            for l in range(L):
                xin = x11[:] if l == L - 1 else xls[l]
                dg = dgpool.tile([P, P], F32R, tag="dg")
                nc.vector.tensor_scalar(out=dg[:], in0=ident[:],
                                        scalar1=wts[:, l:l + 1], scalar2=None,
                                        op0=ALU.mult)
                for nh in range(2):
                    nc.tensor.matmul(
                        pm[:, nh * 512:(nh + 1) * 512],
                        lhsT=dg[:],
                        rhs=xin[:, nh * 512:(nh + 1) * 512],
                        start=(l == 0), stop=(l == L - 1),
                    )
            # copy+store in column halves so the first store overlaps the
            # second copy (shortens the last tile's drain)
            osb = opool.tile([P, D], F32, tag="osb")
            for nh in range(2):
                nc.scalar.copy(osb[:, nh * 512:(nh + 1) * 512],
                               pm[:, nh * 512:(nh + 1) * 512])
                nc.scalar.dma_start(
                    out_dram[r0:r0 + P, nh * 512:(nh + 1) * 512],
                    osb[:, nh * 512:(nh + 1) * 512])

    nc.compile()
    return nc


def _build_program_v1(npc, gate, use_affine, bench_loop=0):
    """v1 program (keys-free rewrite, LN-first).  Kept for the affine path."""
    assert npc % P == 0
    nt = npc // P
    g = float(gate)
    K_ACT_V1 = 5

    nc = bacc.Bacc("TRN2", target_bir_lowering=False, debug=False)

    x_dram = nc.dram_tensor("states_shard", [L, npc, D], F32R, kind="ExternalInput")
    wqt_dram = nc.dram_tensor("wqt", [P, 8 * DK], F32R, kind="ExternalInput")
    wk_dram = nc.dram_tensor("wk", [P, 2 * D], F32R, kind="ExternalInput")
    if use_affine:
        lnw_dram = nc.dram_tensor("lnw", [1, D], F32, kind="ExternalInput")
        lnb_dram = nc.dram_tensor("lnb", [1, D], F32, kind="ExternalInput")
    out_dram = nc.dram_tensor("out", [npc, D], F32, kind="ExternalOutput")

    with tile.TileContext(nc) as tc, ExitStack() as ctx:
        cpool = ctx.enter_context(tc.tile_pool(name="consts", bufs=1))
        gpool = ctx.enter_context(tc.tile_pool(name="globals", bufs=1))
        xpool = ctx.enter_context(tc.tile_pool(name="x", bufs=2))
        n11pool = ctx.enter_context(tc.tile_pool(name="n11", bufs=2))
        scpool = ctx.enter_context(tc.tile_pool(name="prod", bufs=5))
        bpool = ctx.enter_context(tc.tile_pool(name="dump", bufs=3))
        spool = ctx.enter_context(tc.tile_pool(name="stats", bufs=2))
        dgpool = ctx.enter_context(tc.tile_pool(name="dg", bufs=2))
        pT = ctx.enter_context(tc.tile_pool(name="psum_T", bufs=1, space="PSUM"))
        pQ = ctx.enter_context(tc.tile_pool(name="psum_q", bufs=1, space="PSUM"))
        pU = ctx.enter_context(tc.tile_pool(name="psum_u", bufs=1, space="PSUM"))
        pM = ctx.enter_context(tc.tile_pool(name="psum_m", bufs=2, space="PSUM"))

        ident_f = cpool.tile([P, P], F32)
        masks.make_identity(nc, ident_f[:])
        ident = cpool.tile([P, P], F32R)
        nc.scalar.copy(ident[:], ident_f[:])
        wqt = cpool.tile([P, 8 * DK], F32R)
        nc.sync.dma_start(wqt[:], wqt_dram[:])
        wk = cpool.tile([P, 2 * D], F32R)
        nc.sync.dma_start(wk[:], wk_dram[:])
        if use_affine:
            lnw_bc = cpool.tile([P, D], F32)
            nc.sync.dma_start(lnw_bc[0:1, :], lnw_dram[:])
            nc.gpsimd.partition_broadcast(lnw_bc[:], lnw_bc[0:1, :])
            lnb_bc = cpool.tile([P, D], F32)
            nc.sync.dma_start(lnb_bc[0:1, :], lnb_dram[:])
            nc.gpsimd.partition_broadcast(lnb_bc[:], lnb_bc[0:1, :])

        loop_ctx = tc.For_i(0, bench_loop, 1) if bench_loop > 0 else None
        if loop_ctx is not None:
            ctx.enter_context(loop_ctx)

        x11 = gpool.tile([P, nt, D], F32R)
        n11t = gpool.tile([P, nt, D], F32R)
        u_all = gpool.tile([P, nt, D], F32)
        qsb = gpool.tile([P, 2, nt * P], F32R)
        st_all = gpool.tile([P, nt, L, 12], F32)
        ag_all = gpool.tile([P, nt, L, 2], F32)
        acol_all = gpool.tile([P, nt, L], F32)
        sx_all = gpool.tile([P, nt, L], F32)
        sxx_all = gpool.tile([P, nt, L], F32)
        c1_all = gpool.tile([P, nt], F32)
        if use_affine:
            c2_all = gpool.tile([P, nt], F32)

        with tc.high_priority():
            for t in range(nt):
                for hh in range(2):
                    nc.sync.dma_start(
                        x11[:, t, hh * 512:(hh + 1) * 512],
                        x_dram[L - 1, t * P:(t + 1) * P, hh * 512:(hh + 1) * 512])
            for t in range(nt):
                nc.vector.bn_stats(st_all[:, t, L - 1, 0:6], x11[:, t, 0:512].bitcast(F32))
                nc.vector.bn_stats(st_all[:, t, L - 1, 6:12], x11[:, t, 512:1024].bitcast(F32))
                nc.vector.bn_aggr(ag_all[:, t, L - 1, :], st_all[:, t, L - 1, :])
            vpe11 = spool.tile([P, nt], F32, tag="vpe11")
            nc.vector.tensor_scalar(out=vpe11[:], in0=ag_all[:, :, L - 1, 1],
                                    scalar1=LN_EPS, scalar2=None, op0=ALU.add)
            r11 = gpool.tile([P, nt], F32)
            _rsqrt_newton(nc, spool, vpe11, r11, nt)
            negmur = gpool.tile([P, nt], F32)
            nc.vector.tensor_tensor(out=negmur[:], in0=ag_all[:, :, L - 1, 0],
                                    in1=r11[:], op=ALU.mult)
            nc.vector.tensor_scalar(out=negmur[:], in0=negmur[:], scalar1=-1.0,
                                    scalar2=None, op0=ALU.mult)
            for t in range(nt):
                n11 = n11pool.tile([P, D], F32R, tag="n11")
                nc.vector.tensor_scalar(
                    out=n11[:], in0=x11[:, t, :].bitcast(F32), scalar1=r11[:, t:t + 1],
                    scalar2=negmur[:, t:t + 1], op0=ALU.mult, op1=ALU.add,
                )
                if use_affine:
                    nc.vector.tensor_tensor(out=n11[:], in0=n11[:].bitcast(F32), in1=lnw_bc[:],
                                            op=ALU.mult)
                    nc.vector.tensor_tensor(out=n11[:], in0=n11[:].bitcast(F32), in1=lnb_bc[:],
                                            op=ALU.add)
                for half in range(2):
                    pt = pT.tile([P, 512], F32R, tag="pT")
                    for cc in range(4):
                        c = half * 4 + cc
                        nc.tensor.transpose(
                            pt[:, cc * P:(cc + 1) * P], n11[:, c * P:(c + 1) * P],
                            ident[:])
                    nc.scalar.copy(n11t[:, t, half * 512:(half + 1) * 512], pt[:])
            for tp in range((nt + 1) // 2):
                tw = min(2, nt - tp * 2)
                for h in range(2):
                    pq = pQ.tile([P, 2 * P], F32, tag="pq")
                    for c in range(8):
                        nc.tensor.matmul(
                            pq[:, 0:tw * P],
                            lhsT=wqt[:, c * DK + h * P: c * DK + (h + 1) * P]
                                .bitcast(F32R),
                            rhs=n11t[:, tp * 2:tp * 2 + tw, c * P:(c + 1) * P]
                                .bitcast(F32R),
                            start=(c == 0), stop=(c == 7),
                        )
                    nc.scalar.copy(qsb[:, h, tp * 2 * P:(tp * 2 + tw) * P],
                                   pq[:, 0:tw * P])
            for t in range(nt):
                pu = pU.tile([P, D], F32, tag="pu")
                for h in range(2):
                    for nh in range(2):
                        nc.tensor.matmul(
                            pu[:, nh * 512:(nh + 1) * 512],
                            lhsT=qsb[:, h, t * P:(t + 1) * P].bitcast(F32R),
                            rhs=wk[:, h * D + nh * 512: h * D + (nh + 1) * 512]
                                .bitcast(F32R),
                            start=(h == 0), stop=(h == 1),
                        )
                nc.scalar.activation(out=u_all[:, t, :], in_=pu[:],
                                     func=ACTF.Copy,
                                     accum_out=(None if use_affine
                                                else c1_all[:, t:t + 1]))
                if use_affine:
                    scc2 = scpool.tile([P, D], F32, tag="pr")
                    nc.gpsimd.tensor_tensor(out=scc2[:], in0=u_all[:, t, :],
                                            in1=lnb_bc[:], op=ALU.mult)
                    nc.vector.tensor_reduce(out=c2_all[:, t:t + 1], in_=scc2[:],
                                            axis=mybir.AxisListType.X, op=ALU.add)
                    nc.vector.tensor_tensor(out=u_all[:, t, :], in0=u_all[:, t, :],
                                            in1=lnw_bc[:], op=ALU.mult)
                    nc.vector.tensor_reduce(out=c1_all[:, t:t + 1],
                                            in_=u_all[:, t, :],
                                            axis=mybir.AxisListType.X, op=ALU.add)

        for t in range(nt):
            r0 = t * P
            xls = []
            for l in range(L - 1):
                xl = xpool.tile([P, D], F32R, tag="xl")
                nc.sync.dma_start(xl[:], x_dram[l, r0:r0 + P, :])
                xls.append(xl)
            for l in range(L - 1):
                if l < K_ACT_V1:
                    dc = bpool.tile([P, D], BF16, tag="dump")
                    nc.scalar.activation(out=dc[:], in_=xls[l].bitcast(F32),
                                         func=ACTF.Copy,
                                         accum_out=sx_all[:, t, l:l + 1])
                    ds = bpool.tile([P, D], BF16, tag="dump")
                    nc.scalar.activation(out=ds[:], in_=xls[l].bitcast(F32),
                                         func=ACTF.Square,
                                         accum_out=sxx_all[:, t, l:l + 1])
                else:
                    nc.vector.bn_stats(st_all[:, t, l, 0:6],
                                       xls[l][:, 0:512].bitcast(F32))
                    nc.vector.bn_stats(st_all[:, t, l, 6:12],
                                       xls[l][:, 512:1024].bitcast(F32))
                    nc.vector.bn_aggr(ag_all[:, t, l, :], st_all[:, t, l, :])
            if K_ACT_V1:
                tma = spool.tile([P, K_ACT_V1], F32, tag="tma")
                tmb = spool.tile([P, K_ACT_V1], F32, tag="tmb")
                nc.vector.tensor_scalar(out=ag_all[:, t, 0:K_ACT_V1, 0],
                                        in0=sx_all[:, t, 0:K_ACT_V1],
                                        scalar1=1.0 / D, scalar2=None,
                                        op0=ALU.mult)
                nc.vector.tensor_scalar(out=tma[:], in0=sx_all[:, t, 0:K_ACT_V1],
                                        scalar1=1.0 / D, scalar2=None,
                                        op0=ALU.mult)
                nc.vector.tensor_tensor(out=tma[:], in0=tma[:], in1=tma[:],
                                        op=ALU.mult)
                nc.vector.tensor_scalar(out=tmb[:], in0=sxx_all[:, t, 0:K_ACT_V1],
                                        scalar1=1.0 / D, scalar2=None,
                                        op0=ALU.mult)
                nc.vector.tensor_tensor(out=ag_all[:, t, 0:K_ACT_V1, 1],
                                        in0=tmb[:], in1=tma[:],
                                        op=ALU.subtract)
            for l in range(L):
                xin = x11[:, t, :] if l == L - 1 else xls[l][:]
                xin_f = xin.bitcast(F32)
                pr = scpool.tile([P, D], F32, tag="pr")
                nc.vector.scalar_tensor_tensor(
                    out=pr[:], in0=xin_f, scalar=0.0, in1=u_all[:, t, :],
                    op0=ALU.add, op1=ALU.mult,
                    accum_out=acol_all[:, t, l:l + 1])

            vpe = spool.tile([P, L], F32, tag="vpe")
            nc.vector.tensor_scalar(out=vpe[:], in0=ag_all[:, t, :, 1],
                                    scalar1=LN_EPS, scalar2=None, op0=ALU.add)
            rr = spool.tile([P, L], F32, tag="rr")
            _rsqrt_newton(nc, spool, vpe, rr, L)
            lg = spool.tile([P, L], F32, tag="lg")
            nc.vector.tensor_tensor(out=lg[:], in0=acol_all[:, t, :], in1=rr[:],
                                    op=ALU.mult)
            mur = spool.tile([P, L], F32, tag="mur")
            nc.vector.tensor_tensor(out=mur[:], in0=ag_all[:, t, :, 0], in1=rr[:],
                                    op=ALU.mult)
            nc.vector.tensor_scalar(out=mur[:], in0=mur[:],
                                    scalar1=c1_all[:, t:t + 1],
                                    scalar2=None, op0=ALU.mult)
            nc.vector.tensor_tensor(out=lg[:], in0=lg[:], in1=mur[:],
                                    op=ALU.subtract)
            if use_affine:
                nc.vector.tensor_scalar(out=lg[:], in0=lg[:],
                                        scalar1=c2_all[:, t:t + 1],
                                        scalar2=None, op0=ALU.add)
            negmax = spool.tile([P, 1], F32, tag="negmax")
            nc.vector.tensor_reduce(out=negmax[:], in_=lg[:],
                                    axis=mybir.AxisListType.X, op=ALU.max,
                                    negate=True)
            nc.vector.tensor_scalar(out=negmax[:], in0=negmax[:], scalar1=SCALE,
                                    scalar2=None, op0=ALU.mult)
            wts = spool.tile([P, L], F32, tag="wts")
            ssum = spool.tile([P, 1], F32, tag="ssum")
            nc.scalar.activation(
                out=wts[:], in_=lg[:], func=ACTF.Exp, bias=negmax[:], scale=SCALE,
                accum_out=ssum[:],
            )
            rs = spool.tile([P, 1], F32, tag="rs")
            nc.vector.reciprocal(rs[:], ssum[:])
            nc.vector.tensor_scalar(out=rs[:], in0=rs[:], scalar1=(1.0 - g),
                                    scalar2=None, op0=ALU.mult)
            nc.vector.tensor_scalar(out=wts[:], in0=wts[:], scalar1=rs[:],
                                    scalar2=None, op0=ALU.mult)
            nc.vector.tensor_scalar(out=wts[:, L - 1:L], in0=wts[:, L - 1:L],
                                    scalar1=g, scalar2=None, op0=ALU.add)

            pm = pM.tile([P, D], F32, tag="pm")
            for l in range(L):
                xin = x11[:, t, :] if l == L - 1 else xls[l][:]
                dg = dgpool.tile([P, P], F32R, tag="dg")
                nc.vector.tensor_scalar(out=dg[:], in0=ident[:],
                                        scalar1=wts[:, l:l + 1], scalar2=None,
                                        op0=ALU.mult)
                for nh in range(2):
                    nc.tensor.matmul(
                        pm[:, nh * 512:(nh + 1) * 512],
                        lhsT=dg[:],
                        rhs=xin[:, nh * 512:(nh + 1) * 512],
                        start=(l == 0), stop=(l == L - 1),
                    )
            osb = n11pool.tile([P, D], F32, tag="osb")
            nc.scalar.copy(osb[:], pm[:])
            nc.sync.dma_start(out_dram[r0:r0 + P, :], osb[:])

    nc.compile()
    return nc


_PROGRAM_CACHE = {}


def _get_program(npc, gate, use_affine):
    key = (npc, round(float(gate), 10), bool(use_affine))
    if key not in _PROGRAM_CACHE:
        _PROGRAM_CACHE[key] = build_program(npc, gate, use_affine)
    return _PROGRAM_CACHE[key]


def kernel(states, Wq, Wk, ln_weight, ln_bias, latest_gate, **_unused):
    states = np.ascontiguousarray(np.asarray(states, dtype=np.float32))
    Wq = np.asarray(Wq, dtype=np.float32)
    Wk = np.asarray(Wk, dtype=np.float32)
    ln_weight = np.asarray(ln_weight, dtype=np.float32)
    ln_bias = np.asarray(ln_bias, dtype=np.float32)
    gate = 1.0 / (1.0 + math.exp(-float(np.asarray(latest_gate))))

    use_affine = not (np.all(ln_weight == 1.0) and np.all(ln_bias == 0.0))
    nc = _get_program(NPC, gate, use_affine)

    # host-side prep of the (replicated) small params
    wqt = np.ascontiguousarray(
        Wq.T.reshape(8, P, DK).transpose(1, 0, 2).reshape(P, 8 * DK))
    wkr = np.ascontiguousarray(
        Wk.reshape(2, P, D).transpose(1, 0, 2).reshape(P, 2 * D))
    svec = (Wk.T @ Wq.sum(axis=1)).reshape(1, D).astype(np.float32)

    xs = states.reshape(L, NTOT, D)
    in_maps = []
    for c in range(N_CORES):
        m = {
            "states_shard": np.ascontiguousarray(xs[:, c * NPC:(c + 1) * NPC, :]),
            "wqt": wqt,
            "wk": wkr,
        }
        if use_affine:
            m["lnw"] = ln_weight.reshape(1, D)
            m["lnb"] = ln_bias.reshape(1, D)
        else:
            m["svec"] = svec
        in_maps.append(m)

    res = run_bass_kernel_spmd(nc, in_maps, list(range(N_CORES)))
    out = np.concatenate([res.results[c]["out"] for c in range(N_CORES)], axis=0)
    return np.ascontiguousarray(out.reshape(B, S, D).astype(np.float32))
